# revision 1
# baseline (speedup 1.0000x reference)
"""Trainium2 Bass kernel: causal MHSA, last-position output (fp32, N-small matmuls).

The reference returns only out[:, -1, :]; with the causal mask the last query
row attends to everything, so per batch element the whole MHSA collapses to
tiny GEMVs (q_row and M = Wk-contracted-with-q fold on the host, removing the
Wq/Wk transfers and the x@Wk / x@Wv matmuls entirely).  Per-core device cost:
stream x (2MB) + Wv/Wo (1MB) from HBM, ~90 matmuls.  Sharding: pure data
parallel over batch, core b <- batch b, no collectives.

The two big matmuls are emitted in transposed form so the streamed (free) dimension is 8 instead of 512/256 —
fp32 matmul cost scales with the free dim (4 cyc/row), while the 128-col
weight loads ride the separate LDW port:

    scores^T tiles [s,8] = xT_chunk.T @ M_chunk      (lhsT = xT, N=8)
    -> exp lands directly in the [s-part, h] layout the attention matmul
       needs, so the w-transpose stage disappears;
    attn^T chunks [f,8]  = x_chunk.T @ w_tile        (lhsT = x,  N=8)
    -> lands directly in the [f-part, h] layout the Wv matmul needs, so the
       attn_x transpose stage disappears.
    softmax sums via ones[128,1].T @ w_tiles accumulation (partition-dim sum).

Everything is fp32 end-to-end (no fp32r): HW rel err ~1.5e-6.
"""

import numpy as np
from contextlib import ExitStack

import concourse.bass as bass
import concourse.tile as tile
from concourse import bacc, mybir
from concourse.bass_utils import run_bass_kernel_spmd
from concourse.masks import make_identity

B, S, F, PROJ, H, D = 8, 2048, 256, 512, 8, 64
NT = S // 128        # 16 s-tiles
FC = F // 128        # 2 f-chunks
SG = 4               # s-tiles per pipeline group
NG = NT // SG        # 4 groups
f32 = mybir.dt.float32
EXP = mybir.ActivationFunctionType.Exp

_cache = {}


def _build():
    nc = bacc.Bacc("TRN2", target_bir_lowering=False, debug=False, num_devices=B)
    x = nc.dram_tensor("x", [S, F], f32, kind="ExternalInput").ap()
    M = nc.dram_tensor("M", [F, H], f32, kind="ExternalInput").ap()
    Wv = nc.dram_tensor("Wv", [F, PROJ], f32, kind="ExternalInput").ap()
    Wo = nc.dram_tensor("Wo", [PROJ, F], f32, kind="ExternalInput").ap()
    bo = nc.dram_tensor("bo", [FC, 128], f32, kind="ExternalInput").ap()
    # 0/1 selectors for the block-diag recip pattern: bd = A.T @ (B * recip)
    Abd = nc.dram_tensor("Abd", [H, 128], f32, kind="ExternalInput").ap()
    Bbd = nc.dram_tensor("Bbd", [H, 4], f32, kind="ExternalInput").ap()
    out = nc.dram_tensor("out", [F], f32, kind="ExternalOutput").ap()

    with tile.TileContext(nc) as tc, ExitStack() as ctx:
        P = ctx.enter_context(tc.tile_pool(name="persist", bufs=1))
        xtp = ctx.enter_context(tc.tile_pool(name="xtp", bufs=3, space="PSUM"))
        sct = ctx.enter_context(tc.tile_pool(name="sct", bufs=1, space="PSUM"))
        pers = ctx.enter_context(tc.tile_pool(name="pers", bufs=1, space="PSUM"))
        axp = ctx.enter_context(tc.tile_pool(name="axp", bufs=2, space="PSUM"))
        tailp = ctx.enter_context(tc.tile_pool(name="tailp", bufs=1, space="PSUM"))

        ident = P.tile([128, 128], f32)
        ones_col = P.tile([128, 1], f32)
        x_sb = P.tile([128, NT, F], f32)
        xT_sb = P.tile([128, FC, S], f32)
        m_sb = P.tile([128, FC, H], f32)
        wv_sb = P.tile([128, FC, PROJ], f32)
        wo_sb = P.tile([128, 4, F], f32)
        bo_sb = P.tile([1, FC, 128], f32)
        wt_sb = P.tile([128, NT * H], f32)
        srecip = P.tile([H, 1], f32)
        axT_sb = P.tile([128, FC * H], f32)
        abd_sb = P.tile([H, 128], f32)
        bbd_sb = P.tile([H, 4], f32)
        bw_sb = P.tile([H, 4], f32)
        bd_sb = P.tile([128, 4], f32)
        ac_sb = P.tile([128, 4], f32)
        o_sb = P.tile([128, FC], f32)
        dummy = P.tile([1, 1], f32)

        # trigger the ACT Exp table load early, overlapped with DMA
        nc.vector.memset(dummy[:], 0.0)
        nc.scalar.activation(out=dummy[:], in_=dummy[:], func=EXP)
        nc.vector.memset(ones_col[:], 1.0)

        make_identity(nc, ident[:])

        # ---- DMAs: x group 0 in halves (earlier compute start), rest of x,
        #      tiny M between, tail weights
        xr = x.rearrange("(t p) f -> p t f", p=128)
        nc.sync.dma_start(out=x_sb[:, 0:2, :], in_=xr[:, 0:2, :])
        nc.sync.dma_start(out=x_sb[:, 2:SG, :], in_=xr[:, 2:SG, :])
        nc.sync.dma_start(out=x_sb[:, 4:6, :], in_=xr[:, 4:6, :])
        nc.sync.dma_start(out=x_sb[:, 6:8, :], in_=xr[:, 6:8, :])
        nc.sync.dma_start(out=m_sb[:], in_=M.rearrange("(c p) h -> p c h", p=128))
        nc.sync.dma_start(out=x_sb[:, 8:10, :], in_=xr[:, 8:10, :])
        nc.sync.dma_start(out=x_sb[:, 10:12, :], in_=xr[:, 10:12, :])
        nc.sync.dma_start(out=x_sb[:, 12:14, :], in_=xr[:, 12:14, :])
        nc.sync.dma_start(out=x_sb[:, 14:16, :], in_=xr[:, 14:16, :])
        nc.sync.dma_start(out=wv_sb[:], in_=Wv.rearrange("(c p) n -> p c n", p=128))
        nc.sync.dma_start(out=wo_sb[:], in_=Wo.rearrange("(c p) n -> p c n", p=128))
        nc.sync.dma_start(out=bo_sb[0:1, :, :], in_=bo[:])
        nc.sync.dma_start(out=abd_sb[:], in_=Abd[:])
        nc.sync.dma_start(out=bbd_sb[:], in_=Bbd[:])

        # ---- PE warm-up: open the HAM clock gate while DMA streams
        warm_ps = xtp.tile([128, SG * 128], f32, tag="xt")
        for j in range(8):
            nc.tensor.transpose(
                warm_ps[:, (j % SG) * 128 : (j % SG + 1) * 128], ident[:], ident[:]
            )

        # persistent PSUM accumulators
        sums_ps = pers.tile([H, 1], f32, tag="sums")
        axc_ps = [
            pers.tile([128, H], f32, tag=f"axc{c}", name=f"axc_ps{c}") for c in range(FC)
        ]

        # ---- software-pipelined emission: transposes run two groups ahead of
        #      scores/attention so the in-order PE stream never stalls on the
        #      DVE copies or the ACT exp of the current group
        def emit_transposes(g):
            lo = g * SG * 128
            for c in range(FC):
                xt_ps = xtp.tile([128, SG * 128], f32, tag="xt", name=f"xt_ps_{g}_{c}")
                for j in range(SG):
                    nc.tensor.transpose(
                        xt_ps[:, j * 128 : (j + 1) * 128],
                        x_sb[:, g * SG + j, c * 128 : (c + 1) * 128],
                        ident[:],
                    )
                nc.vector.tensor_copy(xT_sb[:, c, lo : lo + SG * 128], xt_ps[:])

        def emit_scores_exp(g):
            lo = g * SG * 128
            sct_ps = sct.tile([128, SG * H], f32, tag="sc", name=f"sct_ps_{g}")
            for j in range(SG):
                for c in range(FC):
                    nc.tensor.matmul(
                        sct_ps[:, j * H : (j + 1) * H],
                        xT_sb[:, c, lo + j * 128 : lo + (j + 1) * 128],
                        m_sb[:, c, :],
                        start=(c == 0),
                        stop=(c == FC - 1),
                    )
            nc.scalar.activation(
                out=wt_sb[:, g * SG * H : (g + 1) * SG * H],
                in_=sct_ps[:],
                func=EXP,
                scale=0.125,
            )

        def emit_attn(g):
            for j in range(SG):
                t_idx = g * SG + j
                nc.tensor.matmul(
                    sums_ps[:],
                    wt_sb[:, t_idx * H : (t_idx + 1) * H],
                    ones_col[:],
                    start=(t_idx == 0),
                    stop=(t_idx == NT - 1),
                    skip_group_check=True,
                )
                for c in range(FC):
                    nc.tensor.matmul(
                        axc_ps[c][:],
                        x_sb[:, t_idx, c * 128 : (c + 1) * 128],
                        wt_sb[:, t_idx * H : (t_idx + 1) * H],
                        start=(t_idx == 0),
                        stop=(t_idx == NT - 1),
                        skip_group_check=True,
                    )

        emit_transposes(0)
        emit_transposes(1)
        for g in range(NG):
            emit_scores_exp(g)
            if g + 2 < NG:
                emit_transposes(g + 2)
            emit_attn(g)

        # ---- softmax denominator: reciprocal straight off the PSUM column,
        #      then the block-diag recip pattern bd[j, c] = recip[2c + (j>=64)]
        #      via one matmul — emitted BEFORE the attn^T copies so the bd
        #      matmul fills the PE idle slot while DVE moves attn^T to SBUF
        nc.vector.reciprocal(srecip[:], sums_ps[:])
        nc.vector.tensor_scalar_mul(bw_sb[:], bbd_sb[:], srecip[:])
        bd_ps = tailp.tile([128, 4], f32, tag="tail")
        nc.tensor.matmul(bd_ps[:], abd_sb[:], bw_sb[:], start=True, stop=True)
        nc.vector.tensor_copy(bd_sb[:], bd_ps[:])

        # ---- attn^T to SBUF (already in [f-part, h] layout for the Wv matmul)
        for c in range(FC):
            nc.vector.tensor_copy(axT_sb[:, c * H : (c + 1) * H], axc_ps[c][:])

        # ---- attn_full^T blocks [p-part, h]: afT = Wv_block.T @ axT, N=8
        afT_ps = xtp.tile([128, 4 * H], f32, tag="xt")
        for pc in range(4):
            for c in range(FC):
                nc.tensor.matmul(
                    afT_ps[:, pc * H : (pc + 1) * H],
                    wv_sb[:, c, pc * 128 : (pc + 1) * 128],
                    axT_sb[:, c * H : (c + 1) * H],
                    start=(c == 0),
                    stop=(c == FC - 1),
                )
        # afT[j, 8pc+h] = attn_f[h, 128pc+j]; extract col 10c + (j>=64) per chunk,
        # normalizing by the block-diag recip pattern on the way out
        top = afT_ps[0:64, 0:1]
        bot = afT_ps[64:128, 1:2]
        nc.vector.tensor_mul(
            ac_sb[0:64, 0:4],
            bass.AP(tensor=top.tensor, offset=top.offset, ap=[top.ap[0], [10, 4]]),
            bd_sb[0:64, 0:4],
        )
        nc.vector.tensor_mul(
            ac_sb[64:128, 0:4],
            bass.AP(tensor=bot.tensor, offset=bot.offset, ap=[bot.ap[0], [10, 4]]),
            bd_sb[64:128, 0:4],
        )

        # ---- out[256] = attn_col.T @ Wo + bo  (column layout [128, 2]);
        #      bias enters as a rank-1 accumulation, result DMAs out of PSUM
        o_ps = tailp.tile([128, FC], f32, tag="tail")
        for mc in range(FC):
            for c in range(4):
                nc.tensor.matmul(
                    o_ps[:, mc : mc + 1],
                    wo_sb[:, c, mc * 128 : (mc + 1) * 128],
                    ac_sb[:, c : c + 1],
                    start=(c == 0),
                    stop=False,
                    skip_group_check=True,
                )
            nc.tensor.matmul(
                o_ps[:, mc : mc + 1],
                bo_sb[0:1, mc, :],
                ones_col[0:1, 0:1],
                start=False,
                stop=True,
                skip_group_check=True,
            )
        nc.vector.tensor_copy(o_sb[:], o_ps[:])
        nc.sync.dma_start(out=out.rearrange("(c p) -> p c", p=128), in_=o_sb[:])

    nc.compile()
    return nc


def get_nc():
    if "nc" not in _cache:
        _cache["nc"] = _build()
    return _cache["nc"]


def host_prep(inputs: dict) -> list[dict]:
    """Per-core input maps: x slice + host-folded M + shared Wv/Wo/bo."""
    xs = np.ascontiguousarray(np.asarray(inputs["x"], dtype=np.float32))
    Wq = np.asarray(inputs["Wq"], dtype=np.float32)
    Wk = np.asarray(inputs["Wk"], dtype=np.float32)
    shared = {
        k: np.ascontiguousarray(np.asarray(inputs[k], dtype=np.float32))
        for k in ("Wv", "Wo")
    }
    shared["bo"] = np.ascontiguousarray(
        np.asarray(inputs["bo"], dtype=np.float32).reshape(FC, 128)
    )
    j = np.arange(128)
    h = np.arange(H)
    shared["Abd"] = np.ascontiguousarray(
        ((h[:, None] % 2) == (j[None, :] >= 64)).astype(np.float32)
    )
    shared["Bbd"] = np.ascontiguousarray(
        ((h[:, None] // 2) == np.arange(4)[None, :]).astype(np.float32)
    )
    in_maps = []
    for b in range(B):
        q_row = xs[b, -1] @ Wq                                   # [512]
        Mb = (Wk * q_row[None, :]).reshape(F, H, D).sum(-1)      # [256, 8]
        in_maps.append({"x": xs[b], "M": np.ascontiguousarray(Mb), **shared})
    return in_maps


def run_hw(inputs: dict) -> np.ndarray:
    nc = get_nc()
    res = run_bass_kernel_spmd(nc, host_prep(inputs), list(range(B)))
    return np.stack([res.results[b]["out"] for b in range(B)])


def kernel(**inputs) -> np.ndarray:
    return run_hw(inputs)



# revision 4
# speedup vs baseline: 1.3391x; 1.3391x over previous
"""Trainium2 Bass kernel: causal MHSA, last-position output (bf16 datapath).

The reference returns only out[:, -1, :]; with the causal mask the last query
row attends to everything, so per batch element the whole MHSA collapses to
tiny GEMVs.  q_row and M = Wk-contracted-with-q fold on the host (as in the
fp32 baseline); everything that streams through the device is cast to bf16 on
the host, which halves HBM traffic and runs the PE at 1 cyc/row instead of 4.

Sharding: pure data parallel over batch, core b <- batch b, no collectives.

Device-side structure per core:
  - 5 HWDGE DMAs total: x in 4 chunks (chunk 0 also carries the folded M and
    bo packed into extra columns) and one packed Wv|Wo tensor.  Fewer, larger
    DMAs matter because descriptor-gen serializes at ~650ns per DMA on the
    shared HWDGE device.
  - per chunk: PE transposes x tiles (bf16, 1cyc/row) -> PSUM, copy back to
    SBUF (DVE), scores matmuls (xT stationary, M moving, N=8), ACT exp into
    bf16 wt, attention matmuls (x stationary, wt moving, N=8) accumulating
    axc in PSUM, and softmax sums accumulated in the SAME PSUM tile in the
    block-diag [128,4] layout via 64-row tile_position matmuls (even/odd
    head columns), which makes the final normalize a plain elementwise mul.
  - tail: reciprocal of the sums block, axc -> bf16, per-head 64-row Wv
    matmuls directly into the extracted [128,4] layout (tile_position),
    normalize, Wo matmuls, bias add, store via SWDGE prepare/trigger
    (descriptor-gen off the critical path; saves ~1.2us vs a plain HWDGE
    store issued after the data is ready).
"""

import numpy as np
from contextlib import ExitStack

import concourse.bass as bass
import concourse.tile as tile
from concourse import bacc, mybir
from concourse.bass_utils import run_bass_kernel_spmd
from concourse.masks import make_identity

B, S, F, PROJ, H, D = 8, 2048, 256, 512, 8, 64
NT = S // 128         # 16 s-tiles
FC = F // 128         # 2 f-chunks
CHUNKS = (4, 4, 4, 4)  # tiles per x DMA
NCH = len(CHUNKS)
AUX = 18              # extra cols on chunk0: M (16) + bo (2)
f32 = mybir.dt.float32
bf16 = mybir.dt.bfloat16
i32 = mybir.dt.int32
EXP = mybir.ActivationFunctionType.Exp

N_WARM = 20           # PE warm-up transposes (p-state ramp)
USE_KV_STORE = False  # SWDGE prepare/trigger store vs plain HWDGE store

_cache = {}

_CH_OFF = [sum(CHUNKS[:g]) for g in range(NCH)]  # first tile of each chunk


def _build():
    nc = bacc.Bacc("TRN2", target_bir_lowering=False, debug=False, num_devices=B)
    x_dram = []
    for g, ct in enumerate(CHUNKS):
        cols = ct * F + (AUX if g == 0 else 0)
        x_dram.append(nc.dram_tensor(f"x{g}", [128, cols], bf16, kind="ExternalInput").ap())
    w_dram = nc.dram_tensor("W", [128, 2048], bf16, kind="ExternalInput").ap()
    if USE_KV_STORE:
        out = nc.dram_tensor("out", [1, 128, FC, 1], f32, kind="ExternalOutput").ap()
    else:
        out = nc.dram_tensor("out", [128, FC], f32, kind="ExternalOutput").ap()

    with tile.TileContext(nc) as tc, ExitStack() as ctx:
        P = ctx.enter_context(tc.tile_pool(name="persist", bufs=1))
        xtp = ctx.enter_context(tc.tile_pool(name="xtp", bufs=2, space="PSUM"))
        sct = ctx.enter_context(tc.tile_pool(name="sct", bufs=2, space="PSUM"))
        pers = ctx.enter_context(tc.tile_pool(name="pers", bufs=1, space="PSUM"))
        warmp = ctx.enter_context(tc.tile_pool(name="warmp", bufs=1, space="PSUM"))
        tailp = ctx.enter_context(tc.tile_pool(name="tailp", bufs=1, space="PSUM"))

        ident = P.tile([128, 128], bf16)
        ones64 = P.tile([128, 64], bf16)
        scratch = P.tile([128, 128], bf16)
        x_sb = [P.tile([128, ct * F + (AUX if g == 0 else 0)], bf16, name=f"x_sb{g}")
                for g, ct in enumerate(CHUNKS)]
        xT_sb = P.tile([128, FC, S], bf16)
        wt_sb = P.tile([128, NT * H], bf16)
        axT_sb = P.tile([128, FC * H], bf16)
        bd_sb = P.tile([128, 4], f32)
        ac_sb = P.tile([128, 4], bf16)
        o_sb = P.tile([128, FC], f32)
        w_sb = P.tile([128, 2048], bf16)
        if USE_KV_STORE:
            idx_sb = P.tile([128, 1], i32)

        def xv(t, c):
            """x tile t, f-chunk c as a [128,128] SBUF view ([s-part, f])."""
            g = 0
            while g + 1 < NCH and t >= _CH_OFF[g + 1]:
                g += 1
            j = t - _CH_OFF[g]
            base = j * F + c * 128
            return x_sb[g][:, base : base + 128]

        def mv(c):
            return x_sb[0][:, CHUNKS[0] * F + c * H : CHUNKS[0] * F + (c + 1) * H]

        bo_v = x_sb[0][:, CHUNKS[0] * F + 2 * H : CHUNKS[0] * F + 2 * H + FC]

        # ---- init (gpsimd builds constants; DVE memsets warm-up scratch)
        nc.vector.memset(scratch[:], 1.0)
        make_identity(nc, ident[:])
        nc.gpsimd.memset(ones64[:], 1.0)
        if USE_KV_STORE:
            nc.gpsimd.memset(idx_sb[:], 0)

        # ---- store descriptors prepared up-front (SWDGE ring); the trigger
        #      at the end fires them after o_sb is written
        if USE_KV_STORE:
            kv_sem = nc.alloc_semaphore("kv_done")
            nc.gpsimd.kv_writeback(
                out_ap=out,
                in_ap=o_sb[:].rearrange("p (a b c) -> p a b c", b=1, c=1),
                ctx_idxs_ap=idx_sb[:],
                prepare_only=True,
                sem=kv_sem,
            )

        # ---- DMAs (SP/HWDGE): x chunks then weights
        for g in range(NCH):
            nc.sync.dma_start(out=x_sb[g][:], in_=x_dram[g])
        nc.sync.dma_start(out=w_sb[:], in_=w_dram)

        def wv_v(fc, c, half):
            return w_sb[:, fc * 512 + c * 128 + 64 * half : fc * 512 + c * 128 + 64 * half + 64]

        def wo_v(c, mc):
            return w_sb[:, 1024 + c * 256 + mc * 128 : 1024 + c * 256 + (mc + 1) * 128]

        # ---- PE warm-up (p-state ramp; content is irrelevant)
        warm_ps = warmp.tile([128, 128], bf16, tag="warm")
        for _ in range(N_WARM):
            nc.tensor.transpose(warm_ps[:], scratch[:], scratch[:])

        # persistent PSUM accumulators: axc in cols 0..16, sums in 16..20
        axs_ps = pers.tile([128, FC * H + 4], f32, tag="axs")

        def emit_transposes(g):
            xt_ps = xtp.tile([128, FC, CHUNKS[g] * 128], bf16, tag="xt", name=f"xt{g}")
            for j in range(CHUNKS[g]):
                t = _CH_OFF[g] + j
                for c in range(FC):
                    nc.tensor.transpose(
                        xt_ps[:, c, j * 128 : (j + 1) * 128], xv(t, c), ident[:]
                    )
            return xt_ps

        def emit_copy(g, xt_ps):
            lo = _CH_OFF[g] * 128
            nc.vector.tensor_copy(
                xT_sb[:, :, lo : lo + CHUNKS[g] * 128], xt_ps[:]
            )

        def emit_scores(g):
            sct_ps = sct.tile([128, CHUNKS[g] * H], f32, tag="sc", name=f"sc{g}")
            for j in range(CHUNKS[g]):
                t = _CH_OFF[g] + j
                for c in range(FC):
                    nc.tensor.matmul(
                        sct_ps[:, j * H : (j + 1) * H],
                        xT_sb[:, c, t * 128 : (t + 1) * 128],
                        mv(c),
                        start=(c == 0),
                        stop=(c == FC - 1),
                    )
            return sct_ps

        def emit_exp(g, sct_ps):
            nc.scalar.activation(
                out=wt_sb[:, _CH_OFF[g] * H : (_CH_OFF[g] + CHUNKS[g]) * H],
                in_=sct_ps[:],
                func=EXP,
                scale=0.125,
            )

        def emit_attn(g):
            for j in range(CHUNKS[g]):
                t = _CH_OFF[g] + j
                wtt = wt_sb[:, t * H : (t + 1) * H]
                for c in range(FC):
                    nc.tensor.matmul(
                        axs_ps[:, c * H : (c + 1) * H],
                        xv(t, c),
                        wtt,
                        start=(t == 0),
                        stop=(t == NT - 1),
                        skip_group_check=True,
                    )
                # softmax sums accumulated directly in the block-diag [128,4]
                # layout: rows 0:64 <- even heads, rows 64:128 <- odd heads
                for half in range(2):
                    sel = bass.AP(
                        tensor=wtt.tensor, offset=wtt.offset + half, ap=[wtt.ap[0], [2, 4]]
                    )
                    nc.tensor.matmul(
                        axs_ps[64 * half : 64 * (half + 1), FC * H : FC * H + 4],
                        ones64[:],
                        sel,
                        start=(t == 0),
                        stop=(t == NT - 1),
                        skip_group_check=True,
                        tile_position=(0, 64 * half),
                    )

        # ---- software-pipelined emission (per-engine queues are in-order)
        xt0 = emit_transposes(0)
        xt1 = emit_transposes(1)
        emit_copy(0, xt0)
        sc0 = emit_scores(0)
        xt2 = emit_transposes(2)
        emit_copy(1, xt1)
        emit_exp(0, sc0)
        sc1 = emit_scores(1)
        emit_attn(0)
        xt3 = emit_transposes(3)
        emit_copy(2, xt2)
        emit_exp(1, sc1)
        sc2 = emit_scores(2)
        emit_attn(1)
        emit_copy(3, xt3)
        emit_exp(2, sc2)
        sc3 = emit_scores(3)
        emit_attn(2)
        emit_exp(3, sc3)
        emit_attn(3)

        # ---- tail: recip, axc->bf16, per-head Wv matmuls straight into the
        #      extracted [128,4] layout, normalize, Wo matmuls, bias, store
        nc.vector.reciprocal(bd_sb[:], axs_ps[:, FC * H : FC * H + 4])
        nc.vector.tensor_copy(axT_sb[:], axs_ps[:, 0 : FC * H])

        afT_ps = tailp.tile([128, 4], f32, tag="tail", name="afT")
        for c in range(4):
            for half in range(2):
                head = 2 * c + half
                for fc in range(FC):
                    nc.tensor.matmul(
                        afT_ps[64 * half : 64 * (half + 1), c : c + 1],
                        wv_v(fc, c, half),
                        axT_sb[:, fc * H + head : fc * H + head + 1],
                        start=(fc == 0),
                        stop=(fc == FC - 1),
                        skip_group_check=True,
                        tile_position=(0, 64 * half),
                    )
        nc.vector.tensor_mul(ac_sb[:], afT_ps[:], bd_sb[:])

        o_ps = tailp.tile([128, FC], f32, tag="tail2", name="o")
        for mc in range(FC):
            for c in range(4):
                nc.tensor.matmul(
                    o_ps[:, mc : mc + 1],
                    wo_v(c, mc),
                    ac_sb[:, c : c + 1],
                    start=(c == 0),
                    stop=(c == 3),
                    skip_group_check=True,
                )
        nc.vector.tensor_add(o_sb[:], o_ps[:], bo_v)

        if USE_KV_STORE:
            nc.gpsimd.trigger_dma(count=None)
            nc.gpsimd.wait_ge(kv_sem, 16)
        else:
            nc.sync.dma_start(out=out, in_=o_sb[:])

    nc.compile()
    return nc


def get_nc():
    if "nc" not in _cache:
        _cache["nc"] = _build()
    return _cache["nc"]


def host_prep(inputs: dict) -> list[dict]:
    """Per-core input maps: bf16 x chunks (chunk0 carries folded M and bo)
    plus a shared packed bf16 Wv|Wo tensor."""
    import ml_dtypes

    bf = ml_dtypes.bfloat16
    xs = np.asarray(inputs["x"], dtype=np.float32)
    Wq = np.asarray(inputs["Wq"], dtype=np.float32)
    Wk = np.asarray(inputs["Wk"], dtype=np.float32)
    Wv = np.asarray(inputs["Wv"], dtype=np.float32)
    Wo = np.asarray(inputs["Wo"], dtype=np.float32)
    bo = np.asarray(inputs["bo"], dtype=np.float32)

    w_pack = np.concatenate(
        [
            Wv.reshape(FC, 128, PROJ).transpose(1, 0, 2).reshape(128, FC * PROJ),
            Wo.reshape(4, 128, F).transpose(1, 0, 2).reshape(128, 4 * F),
        ],
        axis=1,
    ).astype(bf)
    bo_pack = np.ascontiguousarray(bo.reshape(FC, 128).T)

    in_maps = []
    for b in range(B):
        q_row = xs[b, -1] @ Wq                                   # [512]
        M = (Wk * q_row[None, :]).reshape(F, H, D).sum(-1)       # [256, 8]
        m_pack = M.reshape(FC, 128, H).transpose(1, 0, 2).reshape(128, FC * H)
        flat = xs[b].reshape(NT, 128, F).transpose(1, 0, 2).reshape(128, NT * F)
        m = {"W": w_pack}
        for g, ct in enumerate(CHUNKS):
            lo = _CH_OFF[g] * F
            part = flat[:, lo : lo + ct * F]
            if g == 0:
                part = np.concatenate([part, m_pack, bo_pack], axis=1)
            m[f"x{g}"] = np.ascontiguousarray(part.astype(bf))
        in_maps.append(m)
    return in_maps


def run_hw(inputs: dict) -> np.ndarray:
    nc = get_nc()
    res = run_bass_kernel_spmd(nc, host_prep(inputs), list(range(B)))
    outs = []
    for b in range(B):
        o = np.asarray(res.results[b]["out"], dtype=np.float32).reshape(128, FC)
        outs.append(o.T.reshape(F))
    return np.stack(outs)


def kernel(**inputs) -> np.ndarray:
    return run_hw(inputs)
